# revision 33
# baseline (speedup 1.0000x reference)
"""Self-attention (SAGAN-style) Bass kernel for one TRN2 chip (8 NeuronCores).

Reference (B=4, H=W=64, C=256, D=32, N=H*W=4096):
    xf = x.reshape(B, N, C)
    k = xf @ Wk + bk; q = xf @ Wq + bq; v = xf @ Wv + bv
    attn = softmax(q @ k^T, -1);  out = gamma * (attn@v @ Wo + bo) + xf

Sharding: core i handles batch b=i//2, query-row half h=i%2 (2048 rows);
k/v computed over all 4096 keys on every core.

Host-side exact folds: bk drops out of softmax; v-bias/out-bias/gamma fold
into a constant row bias added on the HOST (the device returns only the
attention delta in bf16; host computes out = x + delta + bias in f32).
bq (zero for this problem) would ride an extra wv column (exact path kept).

Device pipeline per core (engines balanced around the N^2 softmax):
  - S^T per window w of 3|2 m-tiles: 4x-row-packed K=32 matmuls -> PSUM.
  - exp: split across ScalarE (exact activation, ~2/3 of windows) and
    VectorE (Schraudolph bf16 bit-trick via tensor_scalar f32->int16 with
    bitcast, ~1/3 of windows; max rel err ~3%, diluted to ~1e-3 in the
    output by softmax normalization and the gamma*delta+x residual).
  - PV: solo K=128 matmuls accumulate U (33rd 'ones' column carries the
    softmax denominator r) into one PSUM bank per superblock.
  - epilogue per superblock: usb cast, out-proj vs gamma-folded Wo_aug,
    r-reciprocal scaling, bf16 delta DMA out; spread one action per window
    to avoid PE head-of-line stalls.
All projections (q/k/v) are software-pipelined just-in-time during the
ramp and superblock 0 so the exp chain starts ~4us in and never starves;
only ScalarE runs exps (no casts/DMAs on it) to keep its chain dense.
"""
import numpy as np
from collections import deque
from contextlib import ExitStack

import concourse.bass as bass
import concourse.bacc as bacc
import concourse.tile as tile
from concourse import mybir
from concourse import bass_utils

F32 = mybir.dt.float32
BF16 = mybir.dt.bfloat16
I16 = mybir.dt.int16

B, HH, WW, C = 4, 64, 64, 256
N = HH * WW          # 4096 key/value rows
D = 32               # head dim
NCORES = 8
QSH = N // 2         # 2048 query rows per core
SBW = 512            # superblock width (q columns)
NSB = QSH // SBW     # 4 superblocks
NMT = N // 128       # 32 m-tiles
NW = [3] * 10 + [2]  # m-tiles per window (sum = 32)
NWIN = len(NW)
G = NSB * NWIN       # 44 window instances
JSTART = {w: sum(NW[:w]) for w in range(NWIN)}
ts = bass.ts

# Schraudolph bf16-exp constants: bits16(int16) = trunc(x*A + B) read as
# bf16 ~= exp(x).  B centers the piecewise-linear 2^frac error at +-3.3%.
EXP_A = float(np.float32(128.0 / np.log(2.0)))
EXP_B = 16251.0

# windows whose exp runs on the DVE (approx); rest exact on ScalarE
EXP_DVE = frozenset(g for g in range(G) if g % 5 in (2, 4))
PV_LAG = 2           # PV trails exp by 2 windows


def build_graph(with_bq=False):
    nc = bacc.Bacc("TRN2", target_bir_lowering=False, debug=False)

    NV = 33 if with_bq else 32
    WCOLS = 192 + 64 + 2 * NV + 257  # wq3 | wk | wv | wo_aug(rows 0:33)
    xT_d = nc.dram_tensor("xT", [C, N], BF16, kind="ExternalInput").ap()
    wpk_d = nc.dram_tensor("wpk", [128, WCOLS], BF16,
                           kind="ExternalInput").ap()
    out_d = nc.dram_tensor("out", [QSH, C], BF16, kind="ExternalOutput").ap()

    ExpF = mybir.ActivationFunctionType.Exp
    Amul = mybir.AluOpType.mult
    Aadd = mybir.AluOpType.add

    with tile.TileContext(nc) as tc, ExitStack() as ctx:
        persist = ctx.enter_context(tc.tile_pool(name="persist", bufs=1))
        stp = ctx.enter_context(tc.tile_pool(name="stp", bufs=2, space="PSUM"))
        uTp = ctx.enter_context(tc.tile_pool(name="uTp", bufs=1, space="PSUM"))
        miscp = ctx.enter_context(
            tc.tile_pool(name="miscp", bufs=1, space="PSUM"))
        expp = ctx.enter_context(tc.tile_pool(name="expp", bufs=4))
        smallp = ctx.enter_context(tc.tile_pool(name="smallp", bufs=2))

        # ---- persistent SBUF tensors ----
        xt = [[persist.tile([128, 1024], BF16, name=f"xt{t}_{hf}")
               for hf in range(2)] for t in range(4)]
        ost = persist.tile([128, 16 * C], BF16)   # bf16 delta staging
        wpk = persist.tile([128, WCOLS], BF16)    # all weights, one DMA
        wq_sb = wpk[:, 0:192]
        wk_sb = wpk[:, 192:256]
        wv_sb = wpk[:, 256:256 + 2 * NV]
        wo_sb = wpk[0:33, 256 + 2 * NV:256 + 2 * NV + 257]
        qT4 = persist.tile([128, QSH], BF16)      # rows 32r:32r+32, r=0..2
        kT4w = [persist.tile([128, 128], BF16, name=f"kT4w{w}")
                for w in range(NWIN)]
        v4w = [persist.tile([128, 33 * nw], BF16, name=f"v4w{w}")
               for w, nw in enumerate(NW)]
        dummy = persist.tile([1, 1], F32)
        scratch = persist.tile([128, 512], BF16)
        if with_bq:
            kbq_sb = persist.tile([128, NMT], F32)
            ekbq = persist.tile([128, NMT], F32)

        # preload the exp table set while the input DMAs run
        nc.vector.memset(dummy[:], 0.0)
        nc.scalar.activation(dummy[:], dummy[:], ExpF)
        nc.vector.memset(scratch[:], 0.0)

        # ---- input DMAs. One packed weight DMA (the separate 2D weight
        # transfers are latency-bound at ~0.7us each); chunk 0 of xT in
        # 512-col pieces so the first projections start earliest.
        nc.scalar.dma_start(wpk[:], wpk_d)
        nc.sync.dma_start(xt[0][0][:, 0:512], xT_d[0:128, 0:512])
        nc.gpsimd.dma_start(xt[0][1][:, 0:512], xT_d[128:256, 0:512])
        nc.sync.dma_start(xt[0][0][:, 512:1024], xT_d[0:128, 512:1024])
        nc.gpsimd.dma_start(xt[0][1][:, 512:1024], xT_d[128:256, 512:1024])
        nc.sync.dma_start(xt[1][0][:], xT_d[0:128, ts(1, 1024)])
        nc.gpsimd.dma_start(xt[1][1][:], xT_d[128:256, ts(1, 1024)])
        nc.sync.dma_start(xt[2][0][:], xT_d[0:128, ts(2, 1024)])
        nc.gpsimd.dma_start(xt[2][1][:], xT_d[128:256, ts(2, 1024)])
        nc.scalar.dma_start(xt[3][0][:], xT_d[0:128, ts(3, 1024)])
        nc.scalar.dma_start(xt[3][1][:], xT_d[128:256, ts(3, 1024)])
        if not with_bq:
            for vt in v4w:  # col 32 of each block stays 1 (denominator)
                nc.gpsimd.memset(vt[:], 1.0)
        dma_qs = [nc.sync, nc.gpsimd]

        # HAM warm-up: a dense matmul burst from t0 flips the PE clock
        # gate to 2.4 GHz right as the first real projections start; a
        # shorter burst provably leaves the whole ramp at 1.2 GHz.
        for _ in range(8):
            dm = stp.tile([128, 512], F32, tag="st")
            nc.tensor.matmul(dm[:], scratch[:, 0:128], scratch[:],
                             start=True, stop=True)

        def cast(out, in_):
            nc.vector.tensor_copy(out, in_)

        # prologue projection scratch alternates the misc and uT banks
        # (uT bank is free until PV(0) at step 2)
        _ring = [("mp", miscp), ("uT", uTp)]
        _ring_i = [0]

        def proj_tile(shape):
            tag, pool = _ring[_ring_i[0] % 2]
            _ring_i[0] += 1
            return pool.tile(shape, F32, name="pt", tag=tag)

        inloop = [False]

        def issue_qproj_half(s, hq):
            # qT cols s*512+hq*256 : +256 for all 3 PE row-group replicas
            t, half = divmod(s, 2)
            x0, x1 = xt[t]
            lo = half * 512 + hq * 256
            if inloop[0]:
                pq = miscp.tile([96, 256], F32, name="pt", tag="mp")
            else:
                pq = proj_tile([96, 256])
            nc.tensor.matmul(pq[0:96, :], wq_sb[:, 0:96], x0[:, lo:lo + 256],
                             start=True, stop=False)
            nc.tensor.matmul(pq[0:96, :], wq_sb[:, 96:192],
                             x1[:, lo:lo + 256], start=False, stop=True)
            dst = qT4[0:96, s * SBW + hq * 256:s * SBW + hq * 256 + 256]
            if s == 0:  # ramp-critical: ScalarE is idle before exp(0)
                nc.scalar.copy(dst, pq[0:96, :])
            else:
                cast(dst, pq[0:96, :])

        def issue_kproj(w):
            nw = NW[w]
            if inloop[0]:
                pk = miscp.tile([128, 128], F32, name="pt", tag="mp")
            else:
                pk = proj_tile([128, 128])
            for r in range(nw):
                j = JSTART[w] + r
                x0, x1 = xt[j // 8]
                lo = 128 * (j % 8)
                nc.tensor.matmul(pk[32 * r:32 * r + 32, 0:128],
                                 wk_sb[:, 0:32], x0[:, lo:lo + 128],
                                 start=True, stop=False,
                                 skip_group_check=True,
                                 tile_position=(0, 32 * r))
                nc.tensor.matmul(pk[32 * r:32 * r + 32, 0:128],
                                 wk_sb[:, 32:64], x1[:, lo:lo + 128],
                                 start=False, stop=True,
                                 skip_group_check=True,
                                 tile_position=(0, 32 * r))
            cast(kT4w[w][0:32 * nw, :], pk[0:32 * nw, 0:128])

        def issue_vproj(w):
            nw = NW[w]
            if inloop[0]:
                pv = miscp.tile([128, nw * NV], F32, name="pt", tag="mp")
            else:
                pv = proj_tile([128, nw * NV])
            for r in range(nw):
                j = JSTART[w] + r
                x0, x1 = xt[j // 8]
                nc.tensor.matmul(pv[:, r * NV:r * NV + NV],
                                 x0[:, ts(j % 8, 128)], wv_sb[:, 0:NV],
                                 start=True, stop=False)
                nc.tensor.matmul(pv[:, r * NV:r * NV + NV],
                                 x1[:, ts(j % 8, 128)], wv_sb[:, NV:2 * NV],
                                 start=False, stop=True)
            pv3 = pv[:, 0:nw * NV].rearrange("p (n c) -> p n c", c=NV)
            v43 = v4w[w][:, 0:nw * 33].rearrange("p (n c) -> p n c", c=33)
            cast(v43[:, :, 0:32], pv3[:, :, 0:32])
            if with_bq:
                jb = JSTART[w]
                nc.vector.tensor_copy(
                    kbq_sb[:, jb:jb + nw].unsqueeze(2), pv3[:, :, 32:33])
                nc.scalar.activation(ekbq[:, jb:jb + nw],
                                     kbq_sb[:, jb:jb + nw], ExpF)
                for r in range(nw):
                    j = jb + r
                    nc.vector.tensor_scalar(
                        v4w[w][:, 33 * r:33 * r + 32],
                        v4w[w][:, 33 * r:33 * r + 32],
                        ekbq[:, j:j + 1], None, op0=Amul)
                    nc.vector.tensor_copy(
                        v4w[w][:, 33 * r + 32:33 * r + 33],
                        ekbq[:, j:j + 1])

        sts = {}

        def issue_st(g):
            s, w = divmod(g, NWIN)
            nw = NW[w]
            st = stp.tile([128, SBW * nw], F32, tag="st")
            for r in range(nw):
                nc.tensor.matmul(st[:, ts(r, SBW)],
                                 kT4w[w][32 * r:32 * r + 32, :],
                                 qT4[32 * r:32 * r + 32, ts(s, SBW)],
                                 start=True, stop=True,
                                 tile_position=(32 * r, 0))
            sts[g] = st

        exs = {}
        uT_cur = [None]

        def issue_pv(g):
            s, w = divmod(g, NWIN)
            ex = exs.pop(g)
            for r in range(NW[w]):
                j = JSTART[w] + r
                if j == 0:
                    uT_cur[0] = uTp.tile([128, SBW], F32, name="uT",
                                         tag="uT")
                nc.tensor.matmul(uT_cur[0][0:33, :],
                                 v4w[w][:, 33 * r:33 * r + 33],
                                 ex[:, ts(r, SBW)],
                                 start=(j == 0), stop=(j == NMT - 1),
                                 skip_group_check=True)

        pending = deque()

        def queue_epilogue(s):
            uT_s = uT_cur[0]
            state = {}

            def a_usb():
                usb = smallp.tile([33, SBW], BF16, tag="usb")
                nc.vector.tensor_copy(usb[:], uT_s[0:33, :])
                state["usb"] = usb
            pending.append(a_usb)

            def a_q(qb, s=s):
                usb = state["usb"]
                if s == NSB - 1:
                    o_ps = stp.tile([128, C + 1], F32, tag="st")
                else:
                    o_ps = miscp.tile([128, C + 1], F32, name="pt", tag="mp")
                nc.tensor.matmul(o_ps[:], usb[:, ts(qb, 128)], wo_sb[:],
                                 start=True, stop=True)
                recip = smallp.tile([128, 1], F32, tag="recip")
                nc.vector.reciprocal(recip[:], o_ps[:, C:C + 1])
                qi = s * 4 + qb
                nc.vector.tensor_scalar(ost[:, ts(qi, C)], o_ps[:, 0:C],
                                        recip[:], None, op0=Amul)
                dma_qs[qi % 2].dma_start(out_d[ts(qi, 128), :],
                                        ost[:, ts(qi, C)])
            for qb in range(4):
                pending.append(lambda qb=qb: a_q(qb))

        def issue_exp(g):
            s, w = divmod(g, NWIN)
            st = sts.pop(g)
            ex = expp.tile([128, SBW * NW[w]], BF16, tag="ex")
            if g in EXP_DVE:
                nc.vector.tensor_scalar(ex[:].bitcast(I16), st[:],
                                        EXP_A, EXP_B, op0=Amul, op1=Aadd)
            else:
                nc.scalar.activation(ex[:], st[:], ExpF)
            exs[g] = ex

        # ---- prologue: minimal path to exp(0), then fill the PE ----
        issue_qproj_half(0, 0)
        issue_kproj(0)
        issue_qproj_half(0, 1)
        issue_st(0)
        issue_kproj(1)
        issue_st(1)
        issue_qproj_half(1, 0)
        issue_vproj(0)
        issue_qproj_half(1, 1)
        issue_vproj(1)
        issue_qproj_half(2, 0)
        issue_qproj_half(2, 1)
        issue_qproj_half(3, 0)
        issue_qproj_half(3, 1)
        inloop[0] = True

        # ---- main loop: one exp window per step ----
        for g in range(G):
            s, w = divmod(g, NWIN)
            # epilogue actions first on the DVE queue (small), exp after
            if pending:
                pending.popleft()()
            issue_exp(g)
            g2 = g + 2
            if g2 < G:
                s2, w2 = divmod(g2, NWIN)
                if s2 == 0 and w2 >= 2:
                    issue_kproj(w2)
            if s == 0 and w + 2 < NWIN:
                issue_vproj(w + 2)
            gp = g - PV_LAG
            if gp >= 0:
                issue_pv(gp)
                if gp % NWIN == NWIN - 1:
                    queue_epilogue(gp // NWIN)
            if g2 < G:
                issue_st(g2)

        # ---- tail ----
        for gp in (G - 2, G - 1):
            issue_pv(gp)
            if gp % NWIN == NWIN - 1:
                queue_epilogue(gp // NWIN)
        while pending:
            pending.popleft()()

    nc.compile()
    return nc


_NC_CACHE = {}


def _get_nc(with_bq=False):
    if with_bq not in _NC_CACHE:
        _NC_CACHE[with_bq] = build_graph(with_bq)
    return _NC_CACHE[with_bq]


def _bf16(a):
    import ml_dtypes
    return np.ascontiguousarray(np.asarray(a, dtype=np.float32)
                                .astype(ml_dtypes.bfloat16))


def make_in_maps(x, Wk, bk, Wq, bq, Wv, bv, Wo, bo, gamma):
    """Host-side sharding + exact bias/gamma folding.

    Returns (in_maps, with_bq, xr_bias): the device computes only the
    attention delta; host adds x and xr_bias.
    """
    f32 = np.float32
    xf = np.ascontiguousarray(x, dtype=f32).reshape(B, N, C)
    Wk = np.asarray(Wk, dtype=f32)
    Wq = np.asarray(Wq, dtype=f32)
    Wv = np.asarray(Wv, dtype=f32)
    Wo = np.asarray(Wo, dtype=f32)
    bq = np.asarray(bq, dtype=f32)
    bv = np.asarray(bv, dtype=f32)
    bo = np.asarray(bo, dtype=f32)
    g = np.asarray(gamma, dtype=f32)[0]

    with_bq = bool(np.any(bq != 0.0))
    if with_bq:
        wv_in = np.concatenate([Wv, (Wk @ bq)[:, None]], axis=1)  # [C, 33]
    else:
        wv_in = Wv
    wo_aug = np.zeros((33, C + 1), dtype=f32)
    wo_aug[0:32, 0:C] = g * Wo
    wo_aug[32, C] = 1.0
    xr_bias = (g * (bv @ Wo + bo)).astype(f32)

    # one packed weight blob: [128, wq3|wk|wv|wo_aug] with the [C=256]
    # channel dim folded into two 128-row column halves
    NV = wv_in.shape[1]
    wq3 = np.concatenate([Wq, Wq, Wq], axis=1)  # [C, 96]
    WCOLS = 192 + 64 + 2 * NV + 257
    wpk = np.zeros((128, WCOLS), dtype=f32)
    wpk[:, 0:96] = wq3[0:128]
    wpk[:, 96:192] = wq3[128:256]
    wpk[:, 192:224] = Wk[0:128]
    wpk[:, 224:256] = Wk[128:256]
    wpk[:, 256:256 + NV] = wv_in[0:128]
    wpk[:, 256 + NV:256 + 2 * NV] = wv_in[128:256]
    wpk[0:33, 256 + 2 * NV:256 + 2 * NV + 257] = wo_aug
    wpk_b = _bf16(wpk)

    in_maps = []
    for i in range(NCORES):
        b, h = divmod(i, 2)
        own = xf[b, h * QSH:(h + 1) * QSH]
        other = xf[b, (1 - h) * QSH:(2 - h) * QSH]
        xT = np.concatenate([own, other], axis=0).T  # [C, N], own cols first
        in_maps.append({
            "xT": _bf16(xT),
            "wpk": wpk_b,
        })
    return in_maps, with_bq, xr_bias


def gather_out(results, x, xr_bias):
    xf = np.asarray(x, dtype=np.float32).reshape(B, N, C)
    out = np.empty((B, N, C), dtype=np.float32)
    for i in range(NCORES):
        b, h = divmod(i, 2)
        out[b, h * QSH:(h + 1) * QSH] = results[i]["out"]  # bf16 -> f32
    out += xf
    out += xr_bias[None, None, :]
    return out.reshape(B, HH, WW, C).astype(np.asarray(x).dtype, copy=False)


def kernel(x, Wk, bk, Wq, bq, Wv, bv, Wo, bo, gamma, **run_kwargs):
    in_maps, with_bq, xr_bias = make_in_maps(
        x, Wk, bk, Wq, bq, Wv, bv, Wo, bo, gamma)
    nc = _get_nc(with_bq)
    res = bass_utils.run_bass_kernel_spmd(
        nc, in_maps, core_ids=list(range(NCORES)), **run_kwargs
    )
    out = gather_out(res.results, x, xr_bias)
    if run_kwargs:
        return out, res
    return out


# revision 39
# speedup vs baseline: 1.2356x; 1.2356x over previous
"""Self-attention (SAGAN-style) Bass kernel for one TRN2 chip (8 NeuronCores).

Reference (B=4, H=W=64, C=256, D=32, N=H*W=4096):
    xf = x.reshape(B, N, C)
    k = xf @ Wk + bk; q = xf @ Wq + bq; v = xf @ Wv + bv
    attn = softmax(q @ k^T, -1);  out = gamma * (attn@v @ Wo + bo) + xf

Sharding: core i handles batch b=i//2, query-row half h=i%2 (2048 rows);
k/v computed over all 4096 keys on every core.

Host-side exact folds: bk drops out of softmax; v-bias/out-bias/gamma fold
into a constant row bias added on the HOST (the device returns only the
attention delta in bf16; host computes out = x + delta + bias in f32).
bq (zero for this problem) would ride an extra wv column (exact path kept).

Device pipeline per core (engines balanced around the N^2 softmax):
  - S^T per window w of 3|2 m-tiles: 4x-row-packed K=32 matmuls -> PSUM.
  - exp: split across ScalarE (exact activation, ~2/3 of windows) and
    VectorE (Schraudolph bf16 bit-trick via tensor_scalar f32->int16 with
    bitcast, ~1/3 of windows; max rel err ~3%, diluted to ~1e-3 in the
    output by softmax normalization and the gamma*delta+x residual).
  - PV: solo K=128 matmuls accumulate U (33rd 'ones' column carries the
    softmax denominator r) into one PSUM bank per superblock.
  - epilogue per superblock: usb cast, out-proj vs gamma-folded Wo_aug,
    r-reciprocal scaling, bf16 delta DMA out; spread one action per window
    to avoid PE head-of-line stalls.
All projections (q/k/v) are software-pipelined just-in-time during the
ramp and superblock 0 so the exp chain starts ~4us in and never starves;
only ScalarE runs exps (no casts/DMAs on it) to keep its chain dense.
"""
import numpy as np
from collections import deque
from contextlib import ExitStack

import concourse.bass as bass
import concourse.bacc as bacc
import concourse.tile as tile
from concourse import mybir
from concourse import bass_utils

F32 = mybir.dt.float32
BF16 = mybir.dt.bfloat16
I16 = mybir.dt.int16

B, HH, WW, C = 4, 64, 64, 256
N = HH * WW          # 4096 key/value rows
D = 32               # head dim
NCORES = 8
QSH = N // 2         # 2048 query rows per core
SBW = 512            # superblock width (q columns)
NSB = QSH // SBW     # 4 superblocks
NMT = N // 128       # 32 m-tiles
NW = [3] * 10 + [2]  # m-tiles per window (sum = 32)
NWIN = len(NW)
G = NSB * NWIN       # 44 window instances
JSTART = {w: sum(NW[:w]) for w in range(NWIN)}
ts = bass.ts

# Schraudolph bf16-exp constants: bits16(int16) = trunc(x*A + B) read as
# bf16 ~= exp(x).  B centers the piecewise-linear 2^frac error at +-3.3%.
EXP_A = float(np.float32(128.0 / np.log(2.0)))
EXP_B = 16251.0

# windows whose exp runs on the DVE (approx); rest exact on ScalarE
EXP_DVE = frozenset(g for g in range(G) if g % 5 in (2, 4))
PV_LAG = 2           # PV trails exp by 2 windows
PV_COLPAIR = False  # col-paired PV hangs on HW (open dual accum groups)


def build_graph(with_bq=False):
    nc = bacc.Bacc("TRN2", target_bir_lowering=False, debug=False)

    NV = 33 if with_bq else 32
    WCOLS = 192 + 64 + 2 * NV + 257  # wq3 | wk | wv | wo_aug(rows 0:33)
    xT_d = nc.dram_tensor("xT", [C, N], BF16, kind="ExternalInput").ap()
    wpk_d = nc.dram_tensor("wpk", [128, WCOLS], BF16,
                           kind="ExternalInput").ap()
    out_d = nc.dram_tensor("out", [QSH, C], BF16, kind="ExternalOutput").ap()

    ExpF = mybir.ActivationFunctionType.Exp
    Amul = mybir.AluOpType.mult
    Aadd = mybir.AluOpType.add

    with tile.TileContext(nc) as tc, ExitStack() as ctx:
        persist = ctx.enter_context(tc.tile_pool(name="persist", bufs=1))
        stp = ctx.enter_context(tc.tile_pool(name="stp", bufs=2, space="PSUM"))
        uTp = ctx.enter_context(tc.tile_pool(name="uTp", bufs=1, space="PSUM"))
        miscp = ctx.enter_context(
            tc.tile_pool(name="miscp", bufs=1, space="PSUM"))
        expp = ctx.enter_context(tc.tile_pool(name="expp", bufs=4))
        smallp = ctx.enter_context(tc.tile_pool(name="smallp", bufs=2))

        # ---- persistent SBUF tensors ----
        xt = [[persist.tile([128, 1024], BF16, name=f"xt{t}_{hf}")
               for hf in range(2)] for t in range(4)]
        ost = persist.tile([128, 16 * C], BF16)   # bf16 delta staging
        wpk = persist.tile([128, WCOLS], BF16)    # all weights, one DMA
        wq_sb = wpk[:, 0:192]
        wk_sb = wpk[:, 192:256]
        wv_sb = wpk[:, 256:256 + 2 * NV]
        # wo_aug duplicated at rows 0:33 and 64:97 for the col-paired PV
        wo_sb = wpk[0:97, 256 + 2 * NV:256 + 2 * NV + 257]
        qT4 = persist.tile([128, QSH], BF16)      # rows 32r:32r+32, r=0..2
        kT4w = [persist.tile([128, 128], BF16, name=f"kT4w{w}")
                for w in range(NWIN)]
        v4w = [persist.tile([128, 33 * nw], BF16, name=f"v4w{w}")
               for w, nw in enumerate(NW)]
        dummy = persist.tile([1, 1], F32)
        scratch = persist.tile([128, 512], BF16)
        if with_bq:
            kbq_sb = persist.tile([128, NMT], F32)
            ekbq = persist.tile([128, NMT], F32)

        # preload the exp table set while the input DMAs run
        nc.vector.memset(dummy[:], 0.0)
        nc.scalar.activation(dummy[:], dummy[:], ExpF)
        nc.vector.memset(scratch[:], 0.0)

        # ---- input DMAs. One packed weight DMA (the separate 2D weight
        # transfers are latency-bound at ~0.7us each); chunk 0 of xT in
        # 512-col pieces so the first projections start earliest.
        nc.scalar.dma_start(wpk[:], wpk_d)
        nc.sync.dma_start(xt[0][0][:, 0:512], xT_d[0:128, 0:512])
        nc.gpsimd.dma_start(xt[0][1][:, 0:512], xT_d[128:256, 0:512])
        nc.sync.dma_start(xt[0][0][:, 512:1024], xT_d[0:128, 512:1024])
        nc.gpsimd.dma_start(xt[0][1][:, 512:1024], xT_d[128:256, 512:1024])
        nc.sync.dma_start(xt[1][0][:], xT_d[0:128, ts(1, 1024)])
        nc.gpsimd.dma_start(xt[1][1][:], xT_d[128:256, ts(1, 1024)])
        nc.sync.dma_start(xt[2][0][:], xT_d[0:128, ts(2, 1024)])
        nc.gpsimd.dma_start(xt[2][1][:], xT_d[128:256, ts(2, 1024)])
        nc.scalar.dma_start(xt[3][0][:], xT_d[0:128, ts(3, 1024)])
        nc.scalar.dma_start(xt[3][1][:], xT_d[128:256, ts(3, 1024)])
        if not with_bq:
            for vt in v4w:  # col 32 of each block stays 1 (denominator)
                nc.gpsimd.memset(vt[:], 1.0)
        dma_qs = [nc.sync, nc.gpsimd]

        # HAM warm-up: a dense matmul burst from t0 flips the PE clock
        # gate to 2.4 GHz right as the first real projections start; a
        # shorter burst provably leaves the whole ramp at 1.2 GHz.
        for _ in range(8):
            dm = stp.tile([128, 512], F32, tag="st")
            nc.tensor.matmul(dm[:], scratch[:, 0:128], scratch[:],
                             start=True, stop=True)

        def cast(out, in_):
            nc.vector.tensor_copy(out, in_)

        # prologue projection scratch alternates the misc and uT banks
        # (uT bank is free until PV(0) at step 2)
        _ring = [("mp", miscp), ("uT", uTp)]
        _ring_i = [0]

        def proj_tile(shape):
            tag, pool = _ring[_ring_i[0] % 2]
            _ring_i[0] += 1
            return pool.tile(shape, F32, name="pt", tag=tag)

        inloop = [False]

        def issue_qproj_half(s, hq):
            # qT cols s*512+hq*256 : +256 for all 3 PE row-group replicas
            t, half = divmod(s, 2)
            x0, x1 = xt[t]
            lo = half * 512 + hq * 256
            if inloop[0]:
                pq = miscp.tile([96, 256], F32, name="pt", tag="mp")
            else:
                pq = proj_tile([96, 256])
            nc.tensor.matmul(pq[0:96, :], wq_sb[:, 0:96], x0[:, lo:lo + 256],
                             start=True, stop=False)
            nc.tensor.matmul(pq[0:96, :], wq_sb[:, 96:192],
                             x1[:, lo:lo + 256], start=False, stop=True)
            dst = qT4[0:96, s * SBW + hq * 256:s * SBW + hq * 256 + 256]
            if s == 0:  # ramp-critical: ScalarE is idle before exp(0)
                nc.scalar.copy(dst, pq[0:96, :])
            else:
                cast(dst, pq[0:96, :])

        def issue_kproj(w):
            nw = NW[w]
            if inloop[0]:
                pk = miscp.tile([128, 128], F32, name="pt", tag="mp")
            else:
                pk = proj_tile([128, 128])
            for r in range(nw):
                j = JSTART[w] + r
                x0, x1 = xt[j // 8]
                lo = 128 * (j % 8)
                nc.tensor.matmul(pk[32 * r:32 * r + 32, 0:128],
                                 wk_sb[:, 0:32], x0[:, lo:lo + 128],
                                 start=True, stop=False,
                                 skip_group_check=True,
                                 tile_position=(0, 32 * r))
                nc.tensor.matmul(pk[32 * r:32 * r + 32, 0:128],
                                 wk_sb[:, 32:64], x1[:, lo:lo + 128],
                                 start=False, stop=True,
                                 skip_group_check=True,
                                 tile_position=(0, 32 * r))
            cast(kT4w[w][0:32 * nw, :], pk[0:32 * nw, 0:128])

        def vproj_mm(pv, w, r):
            j = JSTART[w] + r
            x0, x1 = xt[j // 8]
            nc.tensor.matmul(pv[:, r * NV:r * NV + NV],
                             x0[:, ts(j % 8, 128)], wv_sb[:, 0:NV],
                             start=True, stop=False)
            nc.tensor.matmul(pv[:, r * NV:r * NV + NV],
                             x1[:, ts(j % 8, 128)], wv_sb[:, NV:2 * NV],
                             start=False, stop=True)

        def vproj_cast(pv, w):
            nw = NW[w]
            pv3 = pv[:, 0:nw * NV].rearrange("p (n c) -> p n c", c=NV)
            v43 = v4w[w][:, 0:nw * 33].rearrange("p (n c) -> p n c", c=33)
            cast(v43[:, :, 0:32], pv3[:, :, 0:32])
            if with_bq:
                jb = JSTART[w]
                nc.vector.tensor_copy(
                    kbq_sb[:, jb:jb + nw].unsqueeze(2), pv3[:, :, 32:33])
                nc.scalar.activation(ekbq[:, jb:jb + nw],
                                     kbq_sb[:, jb:jb + nw], ExpF)
                for r in range(nw):
                    j = jb + r
                    nc.vector.tensor_scalar(
                        v4w[w][:, 33 * r:33 * r + 32],
                        v4w[w][:, 33 * r:33 * r + 32],
                        ekbq[:, j:j + 1], None, op0=Amul)
                    nc.vector.tensor_copy(
                        v4w[w][:, 33 * r + 32:33 * r + 33],
                        ekbq[:, j:j + 1])

        def issue_vproj(w):
            if inloop[0]:
                pv = miscp.tile([128, NW[w] * NV], F32, name="pt", tag="mp")
            else:
                pv = proj_tile([128, NW[w] * NV])
            for r in range(NW[w]):
                vproj_mm(pv, w, r)
            vproj_cast(pv, w)

        sts = {}

        def issue_st(g):
            s, w = divmod(g, NWIN)
            nw = NW[w]
            st = stp.tile([128, SBW * nw], F32, tag="st")
            for r in range(nw):
                nc.tensor.matmul(st[:, ts(r, SBW)],
                                 kT4w[w][32 * r:32 * r + 32, :],
                                 qT4[32 * r:32 * r + 32, ts(s, SBW)],
                                 start=True, stop=True,
                                 tile_position=(32 * r, 0))
            sts[g] = st

        exs = {}
        uT_cur = [None]

        def pv_mm(g, r):
            s, w = divmod(g, NWIN)
            ex = exs[g]
            j = JSTART[w] + r
            if j == 0:
                uT_cur[0] = uTp.tile([128, SBW], F32, name="uT", tag="uT")
            if PV_COLPAIR:
                # even m-tiles accumulate at psum partitions 0:33 (col
                # tile T0), odd at 64:97 (T1): independent column tiles
                p = j % 2
                nc.tensor.matmul(uT_cur[0][64 * p:64 * p + 33, :],
                                 v4w[w][:, 33 * r:33 * r + 33],
                                 ex[:, ts(r, SBW)],
                                 start=(j < 2), stop=(j >= NMT - 2),
                                 skip_group_check=True,
                                 tile_position=(0, 64 * p))
            else:
                nc.tensor.matmul(uT_cur[0][0:33, :],
                                 v4w[w][:, 33 * r:33 * r + 33],
                                 ex[:, ts(r, SBW)],
                                 start=(j == 0), stop=(j == NMT - 1),
                                 skip_group_check=True)

        pending = deque()

        def queue_epilogue(s):
            uT_s = uT_cur[0]
            state = {}

            def a_usb():
                usb = smallp.tile([128, SBW], BF16, tag="usb")
                nc.vector.tensor_copy(usb[0:33, :], uT_s[0:33, :])
                if PV_COLPAIR:
                    nc.vector.tensor_copy(usb[64:97, :], uT_s[64:97, :])
                state["usb"] = usb
            pending.append(a_usb)

            def a_q(qb, s=s):
                usb = state["usb"]
                if s == NSB - 1:
                    o_ps = stp.tile([128, C + 1], F32, tag="st")
                else:
                    o_ps = miscp.tile([128, C + 1], F32, name="pt", tag="mp")
                nc.tensor.matmul(o_ps[:], usb[0:33, ts(qb, 128)],
                                 wo_sb[0:33, :], start=True,
                                 stop=not PV_COLPAIR)
                if PV_COLPAIR:
                    nc.tensor.matmul(o_ps[:], usb[64:97, ts(qb, 128)],
                                     wo_sb[64:97, :], start=False, stop=True)
                recip = smallp.tile([128, 1], F32, tag="recip")
                nc.vector.reciprocal(recip[:], o_ps[:, C:C + 1])
                qi = s * 4 + qb
                nc.vector.tensor_scalar(ost[:, ts(qi, C)], o_ps[:, 0:C],
                                        recip[:], None, op0=Amul)
                dma_qs[qi % 2].dma_start(out_d[ts(qi, 128), :],
                                        ost[:, ts(qi, C)])
            for qb in range(4):
                pending.append(lambda qb=qb: a_q(qb))

        def issue_exp(g):
            s, w = divmod(g, NWIN)
            st = sts.pop(g)
            nw = NW[w]
            ex = expp.tile([128, SBW * nw], BF16, tag="ex")
            if g >= G - 2:  # tail: split across both engines
                hi = SBW * nw
                nc.scalar.activation(ex[:, 0:SBW], st[:, 0:SBW], ExpF)
                nc.vector.tensor_scalar(
                    ex[:, SBW:hi].bitcast(I16), st[:, SBW:hi],
                    EXP_A, EXP_B, op0=Amul, op1=Aadd)
            elif g in EXP_DVE:
                nc.vector.tensor_scalar(ex[:].bitcast(I16), st[:],
                                        EXP_A, EXP_B, op0=Amul, op1=Aadd)
            else:
                nc.scalar.activation(ex[:], st[:], ExpF)
            exs[g] = ex

        # ---- prologue: minimal path to exp(0), then fill the PE ----
        issue_qproj_half(0, 0)
        issue_kproj(0)
        issue_qproj_half(0, 1)
        issue_st(0)
        issue_kproj(1)
        issue_st(1)
        issue_qproj_half(1, 0)
        issue_vproj(0)
        issue_qproj_half(1, 1)
        issue_vproj(1)
        issue_qproj_half(2, 0)
        issue_qproj_half(2, 1)
        issue_qproj_half(3, 0)
        issue_qproj_half(3, 1)
        inloop[0] = True

        # ---- main loop: one exp window per step ----
        for g in range(G):
            s, w = divmod(g, NWIN)
            # epilogue actions first on the DVE queue (small), exp after
            if pending:
                pending.popleft()()
            issue_exp(g)
            g2 = g + 2
            if g2 < G:
                s2, w2 = divmod(g2, NWIN)
                if s2 == 0 and w2 >= 2:
                    issue_kproj(w2)
            gp = g - PV_LAG
            wv2 = w + 2
            do_v = s == 0 and wv2 < NWIN
            if do_v:
                nwv = NW[wv2]
                pvt = miscp.tile([128, nwv * NV], F32, name="pt", tag="mp")
            if gp >= 0:
                # interleave vproj MMs between PV MMs: the 128-col vproj
                # weight loads hide under the 512-col PV streams
                sp, wp = divmod(gp, NWIN)
                for r in range(NW[wp]):
                    pv_mm(gp, r)
                    if do_v and r < nwv:
                        vproj_mm(pvt, wv2, r)
                exs.pop(gp)
                if do_v:
                    vproj_cast(pvt, wv2)
                if gp % NWIN == NWIN - 1:
                    queue_epilogue(gp // NWIN)
            elif do_v:
                for r in range(nwv):
                    vproj_mm(pvt, wv2, r)
                vproj_cast(pvt, wv2)
            if g2 < G:
                issue_st(g2)

        # ---- tail ----
        for gp in (G - 2, G - 1):
            for r in range(NW[divmod(gp, NWIN)[1]]):
                pv_mm(gp, r)
            exs.pop(gp)
            if gp % NWIN == NWIN - 1:
                queue_epilogue(gp // NWIN)
        while pending:
            pending.popleft()()

    nc.compile()
    return nc


_NC_CACHE = {}


def _get_nc(with_bq=False):
    if with_bq not in _NC_CACHE:
        _NC_CACHE[with_bq] = build_graph(with_bq)
    return _NC_CACHE[with_bq]


def _bf16(a):
    import ml_dtypes
    return np.ascontiguousarray(np.asarray(a, dtype=np.float32)
                                .astype(ml_dtypes.bfloat16))


def make_in_maps(x, Wk, bk, Wq, bq, Wv, bv, Wo, bo, gamma):
    """Host-side sharding + exact bias/gamma folding.

    Returns (in_maps, with_bq, xr_bias): the device computes only the
    attention delta; host adds x and xr_bias.
    """
    f32 = np.float32
    xf = np.ascontiguousarray(x, dtype=f32).reshape(B, N, C)
    Wk = np.asarray(Wk, dtype=f32)
    Wq = np.asarray(Wq, dtype=f32)
    Wv = np.asarray(Wv, dtype=f32)
    Wo = np.asarray(Wo, dtype=f32)
    bq = np.asarray(bq, dtype=f32)
    bv = np.asarray(bv, dtype=f32)
    bo = np.asarray(bo, dtype=f32)
    g = np.asarray(gamma, dtype=f32)[0]

    with_bq = bool(np.any(bq != 0.0))
    if with_bq:
        wv_in = np.concatenate([Wv, (Wk @ bq)[:, None]], axis=1)  # [C, 33]
    else:
        wv_in = Wv
    wo_aug = np.zeros((33, C + 1), dtype=f32)
    wo_aug[0:32, 0:C] = g * Wo
    wo_aug[32, C] = 1.0
    xr_bias = (g * (bv @ Wo + bo)).astype(f32)

    # one packed weight blob: [128, wq3|wk|wv|wo_aug] with the [C=256]
    # channel dim folded into two 128-row column halves
    NV = wv_in.shape[1]
    wq3 = np.concatenate([Wq, Wq, Wq], axis=1)  # [C, 96]
    WCOLS = 192 + 64 + 2 * NV + 257
    wpk = np.zeros((128, WCOLS), dtype=f32)
    wpk[:, 0:96] = wq3[0:128]
    wpk[:, 96:192] = wq3[128:256]
    wpk[:, 192:224] = Wk[0:128]
    wpk[:, 224:256] = Wk[128:256]
    wpk[:, 256:256 + NV] = wv_in[0:128]
    wpk[:, 256 + NV:256 + 2 * NV] = wv_in[128:256]
    wpk[0:33, 256 + 2 * NV:256 + 2 * NV + 257] = wo_aug
    wpk[64:97, 256 + 2 * NV:256 + 2 * NV + 257] = wo_aug
    wpk_b = _bf16(wpk)

    in_maps = []
    for i in range(NCORES):
        b, h = divmod(i, 2)
        own = xf[b, h * QSH:(h + 1) * QSH]
        other = xf[b, (1 - h) * QSH:(2 - h) * QSH]
        xT = np.concatenate([own, other], axis=0).T  # [C, N], own cols first
        in_maps.append({
            "xT": _bf16(xT),
            "wpk": wpk_b,
        })
    return in_maps, with_bq, xr_bias


def gather_out(results, x, xr_bias):
    xf = np.asarray(x, dtype=np.float32).reshape(B, N, C)
    out = np.empty((B, N, C), dtype=np.float32)
    for i in range(NCORES):
        b, h = divmod(i, 2)
        out[b, h * QSH:(h + 1) * QSH] = results[i]["out"]  # bf16 -> f32
    out += xf
    out += xr_bias[None, None, :]
    return out.reshape(B, HH, WW, C).astype(np.asarray(x).dtype, copy=False)


def kernel(x, Wk, bk, Wq, bq, Wv, bv, Wo, bo, gamma, **run_kwargs):
    in_maps, with_bq, xr_bias = make_in_maps(
        x, Wk, bk, Wq, bq, Wv, bv, Wo, bo, gamma)
    nc = _get_nc(with_bq)
    res = bass_utils.run_bass_kernel_spmd(
        nc, in_maps, core_ids=list(range(NCORES)), **run_kwargs
    )
    out = gather_out(res.results, x, xr_bias)
    if run_kwargs:
        return out, res
    return out


# revision 41
# speedup vs baseline: 1.2807x; 1.0365x over previous
"""Self-attention (SAGAN-style) Bass kernel for one TRN2 chip (8 NeuronCores).

Reference (B=4, H=W=64, C=256, D=32, N=H*W=4096):
    xf = x.reshape(B, N, C)
    k = xf @ Wk + bk; q = xf @ Wq + bq; v = xf @ Wv + bv
    attn = softmax(q @ k^T, -1);  out = gamma * (attn@v @ Wo + bo) + xf

Sharding: core i handles batch b=i//2, query-row half h=i%2 (2048 rows);
k/v computed over all 4096 keys on every core.

Host-side exact folds: bk drops out of softmax; v-bias/out-bias/gamma fold
into a constant row bias added on the HOST (the device returns only the
attention delta in bf16; host computes out = x + delta + bias in f32).
bq (zero for this problem) would ride an extra wv column (exact path kept).

Device pipeline per core (engines balanced around the N^2 softmax):
  - S^T per window w of 3|2 m-tiles: 4x-row-packed K=32 matmuls -> PSUM.
  - exp: split across ScalarE (exact activation, ~2/3 of windows) and
    VectorE (Schraudolph bf16 bit-trick via tensor_scalar f32->int16 with
    bitcast, ~1/3 of windows; max rel err ~3%, diluted to ~1e-3 in the
    output by softmax normalization and the gamma*delta+x residual).
  - PV: solo K=128 matmuls accumulate U (33rd 'ones' column carries the
    softmax denominator r) into one PSUM bank per superblock.
  - epilogue per superblock: usb cast, out-proj vs gamma-folded Wo_aug,
    r-reciprocal scaling, bf16 delta DMA out; spread one action per window
    to avoid PE head-of-line stalls.
All projections (q/k/v) are software-pipelined just-in-time during the
ramp and superblock 0 so the exp chain starts ~4us in and never starves;
only ScalarE runs exps (no casts/DMAs on it) to keep its chain dense.
"""
import numpy as np
from collections import deque
from contextlib import ExitStack

import concourse.bass as bass
import concourse.bacc as bacc
import concourse.tile as tile
from concourse import mybir
from concourse import bass_utils

F32 = mybir.dt.float32
BF16 = mybir.dt.bfloat16
I16 = mybir.dt.int16

B, HH, WW, C = 4, 64, 64, 256
N = HH * WW          # 4096 key/value rows
D = 32               # head dim
NCORES = 8
QSH = N // 2         # 2048 query rows per core
SBW = 512            # superblock width (q columns)
NSB = QSH // SBW     # 4 superblocks
NMT = N // 128       # 32 m-tiles
NW = [3] * 10 + [2]  # m-tiles per window (sum = 32)
NWIN = len(NW)
G = NSB * NWIN       # 44 window instances
JSTART = {w: sum(NW[:w]) for w in range(NWIN)}
ts = bass.ts

# Schraudolph bf16-exp constants: bits16(int16) = trunc(x*A + B) read as
# bf16 ~= exp(x).  B centers the piecewise-linear 2^frac error at +-3.3%.
EXP_A = float(np.float32(128.0 / np.log(2.0)))
EXP_B = 16251.0

# windows whose exp runs on the DVE (approx); rest exact on ScalarE
EXP_DVE = frozenset(g for g in range(G) if g % 5 in (2, 4))
PV_LAG = 2           # PV trails exp by 2 windows
PV_COLPAIR = False  # col-paired PV hangs on HW (open dual accum groups)


def build_graph(with_bq=False):
    nc = bacc.Bacc("TRN2", target_bir_lowering=False, debug=False)

    NV = 33 if with_bq else 32
    WCOLS = 192 + 64 + 2 * NV + 257  # wq3 | wk | wv | wo_aug(rows 0:33)
    xT_d = nc.dram_tensor("xT", [C, N], BF16, kind="ExternalInput").ap()
    wpk_d = nc.dram_tensor("wpk", [128, WCOLS], BF16,
                           kind="ExternalInput").ap()
    out_d = nc.dram_tensor("out", [QSH, C], BF16, kind="ExternalOutput").ap()

    ExpF = mybir.ActivationFunctionType.Exp
    Amul = mybir.AluOpType.mult
    Aadd = mybir.AluOpType.add

    with tile.TileContext(nc) as tc, ExitStack() as ctx:
        persist = ctx.enter_context(tc.tile_pool(name="persist", bufs=1))
        stp = ctx.enter_context(tc.tile_pool(name="stp", bufs=2, space="PSUM"))
        uTp = ctx.enter_context(tc.tile_pool(name="uTp", bufs=1, space="PSUM"))
        miscp = ctx.enter_context(
            tc.tile_pool(name="miscp", bufs=1, space="PSUM"))
        expp = ctx.enter_context(tc.tile_pool(name="expp", bufs=4))
        smallp = ctx.enter_context(tc.tile_pool(name="smallp", bufs=2))

        # ---- persistent SBUF tensors ----
        xt = [[persist.tile([128, 1024], BF16, name=f"xt{t}_{hf}")
               for hf in range(2)] for t in range(4)]
        ost = persist.tile([128, 16 * C], BF16)   # bf16 delta staging
        wpk = persist.tile([128, WCOLS], BF16)    # all weights, one DMA
        wq_sb = wpk[:, 0:192]
        wk_sb = wpk[:, 192:256]
        wv_sb = wpk[:, 256:256 + 2 * NV]
        # wo_aug duplicated at rows 0:33 and 64:97 for the col-paired PV
        wo_sb = wpk[0:97, 256 + 2 * NV:256 + 2 * NV + 257]
        qT4 = persist.tile([128, QSH], BF16)      # rows 32r:32r+32, r=0..2
        kT4w = [persist.tile([128, 128], BF16, name=f"kT4w{w}")
                for w in range(NWIN)]
        v4w = [persist.tile([128, 33 * nw], BF16, name=f"v4w{w}")
               for w, nw in enumerate(NW)]
        dummy = persist.tile([1, 1], F32)
        scratch = persist.tile([128, 512], BF16)
        if with_bq:
            kbq_sb = persist.tile([128, NMT], F32)
            ekbq = persist.tile([128, NMT], F32)

        # preload the exp table set while the input DMAs run
        nc.vector.memset(dummy[:], 0.0)
        nc.scalar.activation(dummy[:], dummy[:], ExpF)
        nc.vector.memset(scratch[:], 0.0)

        # ---- input DMAs. One packed weight DMA (the separate 2D weight
        # transfers are latency-bound at ~0.7us each); chunk 0 of xT in
        # 512-col pieces so the first projections start earliest.
        nc.scalar.dma_start(wpk[:], wpk_d)
        nc.sync.dma_start(xt[0][0][:, 0:512], xT_d[0:128, 0:512])
        nc.gpsimd.dma_start(xt[0][1][:, 0:512], xT_d[128:256, 0:512])
        nc.sync.dma_start(xt[0][0][:, 512:1024], xT_d[0:128, 512:1024])
        nc.gpsimd.dma_start(xt[0][1][:, 512:1024], xT_d[128:256, 512:1024])
        nc.sync.dma_start(xt[1][0][:], xT_d[0:128, ts(1, 1024)])
        nc.gpsimd.dma_start(xt[1][1][:], xT_d[128:256, ts(1, 1024)])
        nc.sync.dma_start(xt[2][0][:], xT_d[0:128, ts(2, 1024)])
        nc.gpsimd.dma_start(xt[2][1][:], xT_d[128:256, ts(2, 1024)])
        nc.scalar.dma_start(xt[3][0][:], xT_d[0:128, ts(3, 1024)])
        nc.scalar.dma_start(xt[3][1][:], xT_d[128:256, ts(3, 1024)])
        if not with_bq:
            for vt in v4w:  # col 32 of each block stays 1 (denominator)
                nc.gpsimd.memset(vt[:], 1.0)
        dma_qs = [nc.sync, nc.gpsimd]

        # HAM warm-up: a dense matmul burst from t0 flips the PE clock
        # gate to 2.4 GHz right as the first real projections start; a
        # shorter burst provably leaves the whole ramp at 1.2 GHz.
        for _ in range(8):
            dm = stp.tile([128, 512], F32, tag="st")
            nc.tensor.matmul(dm[:], scratch[:, 0:128], scratch[:],
                             start=True, stop=True)

        def cast(out, in_):
            nc.vector.tensor_copy(out, in_)

        # prologue projection scratch alternates the misc and uT banks
        # (uT bank is free until PV(0) at step 2)
        _ring = [("mp", miscp), ("uT", uTp)]
        _ring_i = [0]

        def proj_tile(shape):
            tag, pool = _ring[_ring_i[0] % 2]
            _ring_i[0] += 1
            return pool.tile(shape, F32, name="pt", tag=tag)

        inloop = [False]

        def issue_qproj_half(s, hq):
            # qT cols s*512+hq*256 : +256 for all 3 PE row-group replicas
            t, half = divmod(s, 2)
            x0, x1 = xt[t]
            lo = half * 512 + hq * 256
            if inloop[0]:
                pq = miscp.tile([96, 256], F32, name="pt", tag="mp")
            else:
                pq = proj_tile([96, 256])
            nc.tensor.matmul(pq[0:96, :], wq_sb[:, 0:96], x0[:, lo:lo + 256],
                             start=True, stop=False)
            nc.tensor.matmul(pq[0:96, :], wq_sb[:, 96:192],
                             x1[:, lo:lo + 256], start=False, stop=True)
            dst = qT4[0:96, s * SBW + hq * 256:s * SBW + hq * 256 + 256]
            if s == 0:  # ramp-critical: ScalarE is idle before exp(0)
                nc.scalar.copy(dst, pq[0:96, :])
            else:
                cast(dst, pq[0:96, :])

        def issue_kproj(w):
            nw = NW[w]
            if inloop[0]:
                pk = miscp.tile([128, 128], F32, name="pt", tag="mp")
            else:
                pk = proj_tile([128, 128])
            for r in range(nw):
                j = JSTART[w] + r
                x0, x1 = xt[j // 8]
                lo = 128 * (j % 8)
                nc.tensor.matmul(pk[32 * r:32 * r + 32, 0:128],
                                 wk_sb[:, 0:32], x0[:, lo:lo + 128],
                                 start=True, stop=False,
                                 skip_group_check=True,
                                 tile_position=(0, 32 * r))
                nc.tensor.matmul(pk[32 * r:32 * r + 32, 0:128],
                                 wk_sb[:, 32:64], x1[:, lo:lo + 128],
                                 start=False, stop=True,
                                 skip_group_check=True,
                                 tile_position=(0, 32 * r))
            cast(kT4w[w][0:32 * nw, :], pk[0:32 * nw, 0:128])

        def vproj_mm(pv, w, r):
            j = JSTART[w] + r
            x0, x1 = xt[j // 8]
            nc.tensor.matmul(pv[:, r * NV:r * NV + NV],
                             x0[:, ts(j % 8, 128)], wv_sb[:, 0:NV],
                             start=True, stop=False)
            nc.tensor.matmul(pv[:, r * NV:r * NV + NV],
                             x1[:, ts(j % 8, 128)], wv_sb[:, NV:2 * NV],
                             start=False, stop=True)

        def vproj_cast(pv, w):
            nw = NW[w]
            pv3 = pv[:, 0:nw * NV].rearrange("p (n c) -> p n c", c=NV)
            v43 = v4w[w][:, 0:nw * 33].rearrange("p (n c) -> p n c", c=33)
            cast(v43[:, :, 0:32], pv3[:, :, 0:32])
            if with_bq:
                jb = JSTART[w]
                nc.vector.tensor_copy(
                    kbq_sb[:, jb:jb + nw].unsqueeze(2), pv3[:, :, 32:33])
                nc.scalar.activation(ekbq[:, jb:jb + nw],
                                     kbq_sb[:, jb:jb + nw], ExpF)
                for r in range(nw):
                    j = jb + r
                    nc.vector.tensor_scalar(
                        v4w[w][:, 33 * r:33 * r + 32],
                        v4w[w][:, 33 * r:33 * r + 32],
                        ekbq[:, j:j + 1], None, op0=Amul)
                    nc.vector.tensor_copy(
                        v4w[w][:, 33 * r + 32:33 * r + 33],
                        ekbq[:, j:j + 1])

        def issue_vproj(w):
            if inloop[0]:
                pv = miscp.tile([128, NW[w] * NV], F32, name="pt", tag="mp")
            else:
                pv = proj_tile([128, NW[w] * NV])
            for r in range(NW[w]):
                vproj_mm(pv, w, r)
            vproj_cast(pv, w)

        sts = {}

        def issue_st(g):
            s, w = divmod(g, NWIN)
            nw = NW[w]
            st = stp.tile([128, SBW * nw], F32, tag="st")
            for r in range(nw):
                nc.tensor.matmul(st[:, ts(r, SBW)],
                                 kT4w[w][32 * r:32 * r + 32, :],
                                 qT4[32 * r:32 * r + 32, ts(s, SBW)],
                                 start=True, stop=True,
                                 tile_position=(32 * r, 0))
            sts[g] = st

        exs = {}
        uT_cur = [None]

        def pv_mm(g, r):
            s, w = divmod(g, NWIN)
            ex = exs[g]
            j = JSTART[w] + r
            if j == 0:
                uT_cur[0] = uTp.tile([128, SBW], F32, name="uT", tag="uT")
            if PV_COLPAIR:
                # even m-tiles accumulate at psum partitions 0:33 (col
                # tile T0), odd at 64:97 (T1): independent column tiles
                p = j % 2
                nc.tensor.matmul(uT_cur[0][64 * p:64 * p + 33, :],
                                 v4w[w][:, 33 * r:33 * r + 33],
                                 ex[:, ts(r, SBW)],
                                 start=(j < 2), stop=(j >= NMT - 2),
                                 skip_group_check=True,
                                 tile_position=(0, 64 * p))
            else:
                nc.tensor.matmul(uT_cur[0][0:33, :],
                                 v4w[w][:, 33 * r:33 * r + 33],
                                 ex[:, ts(r, SBW)],
                                 start=(j == 0), stop=(j == NMT - 1),
                                 skip_group_check=True)

        pending = deque()

        def queue_epilogue(s):
            uT_s = uT_cur[0]
            state = {}

            def a_usb():
                usb = smallp.tile([128, SBW], BF16, tag="usb")
                nc.vector.tensor_copy(usb[0:33, :], uT_s[0:33, :])
                if PV_COLPAIR:
                    nc.vector.tensor_copy(usb[64:97, :], uT_s[64:97, :])
                state["usb"] = usb
            pending.append(a_usb)

            def a_q(qb, s=s):
                usb = state["usb"]
                if s == NSB - 1:
                    o_ps = stp.tile([128, C + 1], F32, tag="st")
                else:
                    o_ps = miscp.tile([128, C + 1], F32, name="pt", tag="mp")
                nc.tensor.matmul(o_ps[:], usb[0:33, ts(qb, 128)],
                                 wo_sb[0:33, :], start=True,
                                 stop=not PV_COLPAIR)
                if PV_COLPAIR:
                    nc.tensor.matmul(o_ps[:], usb[64:97, ts(qb, 128)],
                                     wo_sb[64:97, :], start=False, stop=True)
                recip = smallp.tile([128, 1], F32, tag="recip")
                nc.vector.reciprocal(recip[:], o_ps[:, C:C + 1])
                qi = s * 4 + qb
                nc.vector.tensor_scalar(ost[:, ts(qi, C)], o_ps[:, 0:C],
                                        recip[:], None, op0=Amul)
                dma_qs[qi % 2].dma_start(out_d[ts(qi, 128), :],
                                        ost[:, ts(qi, C)])
            for qb in range(4):
                pending.append(lambda qb=qb: a_q(qb))

        def issue_exp(g):
            s, w = divmod(g, NWIN)
            st = sts.pop(g)
            nw = NW[w]
            ex = expp.tile([128, SBW * nw], BF16, tag="ex")
            if g >= G - 2:  # tail: split across both engines
                hi = SBW * nw
                nc.scalar.activation(ex[:, 0:SBW], st[:, 0:SBW], ExpF)
                nc.vector.tensor_scalar(
                    ex[:, SBW:hi].bitcast(I16), st[:, SBW:hi],
                    EXP_A, EXP_B, op0=Amul, op1=Aadd)
            elif g in EXP_DVE:
                nc.vector.tensor_scalar(ex[:].bitcast(I16), st[:],
                                        EXP_A, EXP_B, op0=Amul, op1=Aadd)
            else:
                nc.scalar.activation(ex[:], st[:], ExpF)
            exs[g] = ex

        # ---- prologue: minimal path to exp(0), then fill the PE ----
        issue_qproj_half(0, 0)
        issue_kproj(0)
        issue_qproj_half(0, 1)
        issue_st(0)
        issue_kproj(1)
        issue_st(1)
        issue_qproj_half(1, 0)
        issue_vproj(0)
        issue_qproj_half(1, 1)
        issue_vproj(1)
        inloop[0] = True

        # ---- main loop: one exp window per step ----
        for g in range(G):
            s, w = divmod(g, NWIN)
            # epilogue actions first on the DVE queue (small), exp after
            if pending:
                pending.popleft()()
            issue_exp(g)
            g2 = g + 2
            if g2 < G:
                s2, w2 = divmod(g2, NWIN)
                if s2 == 0 and w2 >= 2:
                    issue_kproj(w2)
            # qT for superblocks 2-3 rides superblock 1's PE slack (after
            # epilogue(0) vacates the misc bank; needed at steps 20/31)
            if 17 <= g <= 20:
                issue_qproj_half(2 + (g - 17) // 2, (g - 17) % 2)
            gp = g - PV_LAG
            wv2 = w + 2
            do_v = s == 0 and wv2 < NWIN
            if do_v:
                nwv = NW[wv2]
                pvt = miscp.tile([128, nwv * NV], F32, name="pt", tag="mp")
            if gp >= 0:
                # interleave vproj MMs between PV MMs: the 128-col vproj
                # weight loads hide under the 512-col PV streams
                sp, wp = divmod(gp, NWIN)
                for r in range(NW[wp]):
                    pv_mm(gp, r)
                    if do_v and r < nwv:
                        vproj_mm(pvt, wv2, r)
                exs.pop(gp)
                if do_v:
                    vproj_cast(pvt, wv2)
                if gp % NWIN == NWIN - 1:
                    queue_epilogue(gp // NWIN)
            elif do_v:
                for r in range(nwv):
                    vproj_mm(pvt, wv2, r)
                vproj_cast(pvt, wv2)
            if g2 < G:
                issue_st(g2)

        # ---- tail ----
        for gp in (G - 2, G - 1):
            for r in range(NW[divmod(gp, NWIN)[1]]):
                pv_mm(gp, r)
            exs.pop(gp)
            if gp % NWIN == NWIN - 1:
                queue_epilogue(gp // NWIN)
        while pending:
            pending.popleft()()

    nc.compile()
    return nc


_NC_CACHE = {}


def _get_nc(with_bq=False):
    if with_bq not in _NC_CACHE:
        _NC_CACHE[with_bq] = build_graph(with_bq)
    return _NC_CACHE[with_bq]


def _bf16(a):
    import ml_dtypes
    return np.ascontiguousarray(np.asarray(a, dtype=np.float32)
                                .astype(ml_dtypes.bfloat16))


def make_in_maps(x, Wk, bk, Wq, bq, Wv, bv, Wo, bo, gamma):
    """Host-side sharding + exact bias/gamma folding.

    Returns (in_maps, with_bq, xr_bias): the device computes only the
    attention delta; host adds x and xr_bias.
    """
    f32 = np.float32
    xf = np.ascontiguousarray(x, dtype=f32).reshape(B, N, C)
    Wk = np.asarray(Wk, dtype=f32)
    Wq = np.asarray(Wq, dtype=f32)
    Wv = np.asarray(Wv, dtype=f32)
    Wo = np.asarray(Wo, dtype=f32)
    bq = np.asarray(bq, dtype=f32)
    bv = np.asarray(bv, dtype=f32)
    bo = np.asarray(bo, dtype=f32)
    g = np.asarray(gamma, dtype=f32)[0]

    with_bq = bool(np.any(bq != 0.0))
    if with_bq:
        wv_in = np.concatenate([Wv, (Wk @ bq)[:, None]], axis=1)  # [C, 33]
    else:
        wv_in = Wv
    wo_aug = np.zeros((33, C + 1), dtype=f32)
    wo_aug[0:32, 0:C] = g * Wo
    wo_aug[32, C] = 1.0
    xr_bias = (g * (bv @ Wo + bo)).astype(f32)

    # one packed weight blob: [128, wq3|wk|wv|wo_aug] with the [C=256]
    # channel dim folded into two 128-row column halves
    NV = wv_in.shape[1]
    wq3 = np.concatenate([Wq, Wq, Wq], axis=1)  # [C, 96]
    WCOLS = 192 + 64 + 2 * NV + 257
    wpk = np.zeros((128, WCOLS), dtype=f32)
    wpk[:, 0:96] = wq3[0:128]
    wpk[:, 96:192] = wq3[128:256]
    wpk[:, 192:224] = Wk[0:128]
    wpk[:, 224:256] = Wk[128:256]
    wpk[:, 256:256 + NV] = wv_in[0:128]
    wpk[:, 256 + NV:256 + 2 * NV] = wv_in[128:256]
    wpk[0:33, 256 + 2 * NV:256 + 2 * NV + 257] = wo_aug
    wpk[64:97, 256 + 2 * NV:256 + 2 * NV + 257] = wo_aug
    wpk_b = _bf16(wpk)

    in_maps = []
    for i in range(NCORES):
        b, h = divmod(i, 2)
        own = xf[b, h * QSH:(h + 1) * QSH]
        other = xf[b, (1 - h) * QSH:(2 - h) * QSH]
        xT = np.concatenate([own, other], axis=0).T  # [C, N], own cols first
        in_maps.append({
            "xT": _bf16(xT),
            "wpk": wpk_b,
        })
    return in_maps, with_bq, xr_bias


def gather_out(results, x, xr_bias):
    xf = np.asarray(x, dtype=np.float32).reshape(B, N, C)
    out = np.empty((B, N, C), dtype=np.float32)
    for i in range(NCORES):
        b, h = divmod(i, 2)
        out[b, h * QSH:(h + 1) * QSH] = results[i]["out"]  # bf16 -> f32
    out += xf
    out += xr_bias[None, None, :]
    return out.reshape(B, HH, WW, C).astype(np.asarray(x).dtype, copy=False)


def kernel(x, Wk, bk, Wq, bq, Wv, bv, Wo, bo, gamma, **run_kwargs):
    in_maps, with_bq, xr_bias = make_in_maps(
        x, Wk, bk, Wq, bq, Wv, bv, Wo, bo, gamma)
    nc = _get_nc(with_bq)
    res = bass_utils.run_bass_kernel_spmd(
        nc, in_maps, core_ids=list(range(NCORES)), **run_kwargs
    )
    out = gather_out(res.results, x, xr_bias)
    if run_kwargs:
        return out, res
    return out
